# revision 26
# baseline (speedup 1.0000x reference)
"""Distributed Trainium2 Bass kernel for the DriftingField problem (V2).

Math (reference):
    targets = [gen; pos]                         # [T, D], T = G + P
    d2[i,j] = |gen_i|^2 + |tgt_j|^2 - 2 gen_i.tgt_j
    dist    = sqrt(d2) / sqrt(D); dist[i,i] = 1e6 (gen block diag)
    K       = exp(-dist / TEMP)                  # [G, T]
    nk      = K / sqrt(max(rs_i * cs_j, 1e-12))
    out     = (nk[:,G:] * s_gen) @ pos - (nk[:,:G] * s_pos) @ gen

Numerical fact (same as V1): for this data every K entry ~ exp(-28), so
rs*cs << 1e-12 and the clamp is always active: normalizer == 1e-6 and
    out = 1e12 * [ rg_i * (K[:,G:] @ pos) - rp_i * (K[:,:G] @ gen) ].
G-sharding is embarrassingly parallel; host guard falls back to exact
numpy if the clamp regime is ever left.

V2 layout/schedule (per core, 512 gen rows):
  - One pipelined loop over the 64 j-tiles: PE distance matmuls ->
    DVE (a2/2 - a.b) -> Pool (+diag fill) -> ACT sqrt(bias=b2/2) ->
    ACT exp (col-sum rides accum_out) -> PE output matmuls (first
    512-col half of V) for j-2 interleaved right behind.  No phase
    barrier: PE alternates distance and output matmuls every tile.
  - The "-2" and "/sqrt(D)" scales are folded away by computing
    d2/2 = a2/2 + b2/2 - a.b and dist = sqrt(2)*sqrt(d2/2)/sqrt(D),
    so the matmul operands are RAW bf16 casts (host does layout/dtype
    only: transpose + cast + concat + diag mask).
  - b2/2 per tile: ONE fused DVE tensor_tensor_reduce (square+sum);
    a2/2 from ACT Square(scale=1/sqrt 2) accum during prep.
  - Diagonal fill: +65536 on d2/2's diagonal (exact 0 after exp),
    added by the otherwise-idle Pool engine from a sliding window of a
    per-core host mask (window offset q = T + p - c*RPC is j-tile
    independent).
  - Row sums rg/rp accumulate in PSUM via tiny ones-matmuls sharing
    the output matmuls' stationaries; PSUM exactly fits:
    3 (dist) + 4 (out accum) + 1 (row sums) = 8 banks.
  - Phase 2 streams only V's second 512-col half (8 MB not 32 MB) for
    the remaining output matmuls; dh0 results combine + store early.
"""

import numpy as np
from contextlib import ExitStack

import concourse.bass as bass
import concourse.bacc as bacc
import concourse.mybir as mybir
import concourse.tile as tile
from concourse.bass_utils import run_bass_kernel_spmd

F32 = mybir.dt.float32
BF16 = mybir.dt.bfloat16
AF = mybir.ActivationFunctionType
ALU = mybir.AluOpType

NCORES = 8
TEMP = 0.05
BIGF = 131072.0          # diag fill in d2 domain -> exp underflows to 0
CLAMP = 1.0e-12          # reference: max(rs*cs, 1e-12)
INV_NORM2 = 1.0 / CLAMP  # 1e12, the (1/normalizer)^2 when clamped

TRACE = False
LAST_RESULT = None
DEBUG = False


def build_nc(G, P, D):
    T = G + P
    RPC = G // NCORES          # gen rows per core
    NJ = T // 128              # j-subtiles (target rows)
    NI = RPC // 128            # i-chunks (this core's gen rows)
    ND = D // 128              # d-chunks (feature dim)
    WFILL = T + RPC            # sliding-window fill width
    NJG = G // 128             # j-subtiles in the gen block
    NG = 8                     # j-subtiles per stationary-group DMA
    LAG = 2                    # output-matmul pipeline offset (j-tiles)
    DH = 512                   # output free-dim half width
    EXP_SCALE = -1.0 / (TEMP * float(D) ** 0.5)
    # K = exp(-s*sqrt(d2)) computed as Ln -> Exp -> Exp so every ACT
    # function (ln/exp/copy/square) lives in ONE activation table set
    # (natural_log_exp_and_others) -- no per-tile table reloads:
    #   L = ln(d2 + b2);  mid = exp(0.5*L + ln(s)) = s*sqrt(d2);
    #   K = exp(-mid)
    LN_S = float(np.log(-EXP_SCALE))

    nc = bacc.Bacc(trn_type="TRN2", num_devices=NCORES)

    gen_rows = nc.dram_tensor("gen_rows", [RPC, D], F32, kind="ExternalInput")
    genT_bf = nc.dram_tensor("genT_bf", [D, RPC], BF16, kind="ExternalInput")
    targets_bf = nc.dram_tensor("targets_bf", [T, D], BF16, kind="ExternalInput")
    targets_T_bf = nc.dram_tensor("targets_T_bf", [D, T], BF16,
                                  kind="ExternalInput")
    fill_wide = nc.dram_tensor("fill_wide", [128, WFILL], BF16,
                               kind="ExternalInput")
    out = nc.dram_tensor("out", [RPC, D], F32, kind="ExternalOutput")
    cs_part = nc.dram_tensor("cs_part", [128, NJ], F32, kind="ExternalOutput")
    rs_out = nc.dram_tensor("rs_out", [128, NI], F32, kind="ExternalOutput")
    a2_dram = nc.dram_tensor("a2_dram", [1, RPC], F32)
    if DEBUG:
        b2_dbg = nc.dram_tensor("b2_dbg", [128, NJ], F32, kind="ExternalOutput")
        a2_dbg = nc.dram_tensor("a2_dbg", [128, RPC], F32, kind="ExternalOutput")
        d2_dbg = nc.dram_tensor("d2_dbg", [128, RPC], F32, kind="ExternalOutput")
        dist_dbg = nc.dram_tensor("dist_dbg", [128, RPC], F32,
                                  kind="ExternalOutput")
        kt_dbg = nc.dram_tensor("kt_dbg", [128, RPC], F32, kind="ExternalOutput")

    with tile.TileContext(nc) as tc, ExitStack() as ctx:
        const = ctx.enter_context(tc.tile_pool(name="const", bufs=1))
        work = ctx.enter_context(tc.tile_pool(name="work", bufs=2))

        ones_bf = const.tile([128, 1], BF16, tag="ones_bf")
        nc.vector.memset(ones_bf, 1.0)
        ln_s = const.tile([128, 1], F32, tag="ln_s")
        nc.vector.memset(ln_s, LN_S)

        fill_sb = const.tile([128, WFILL], BF16, tag="fill_sb")
        nc.sync.dma_start(out=fill_sb, in_=fill_wide[:, :])

        b2h = const.tile([128, NJ], F32, tag="b2h")
        cs_sb = const.tile([128, NJ], F32, tag="cs_sb")
        genT = const.tile([128, ND, RPC], BF16, tag="genT")
        a2h_col = const.tile([128, NI], F32, tag="a2h_col")
        a2_row = const.tile([1, RPC], F32, tag="a2_row")
        a2h_bc = const.tile([128, RPC], F32, tag="a2h_bc")
        sgen = const.tile([128, NI, D], F32, tag="sgen")
        spos = const.tile([128, NI, D], F32, tag="spos")
        alpha = const.tile([128, NI], F32, tag="alpha")
        beta = const.tile([128, NI], F32, tag="beta")

        # gen^T straight from DRAM (host-prepped layout), raw bf16
        nc.sync.dma_start(
            out=genT,
            in_=genT_bf[:, :].rearrange("(c p) i -> p c i", p=128))

        # ---- prep: a2 via ACT Square accum + broadcast DMAs ----
        for ic in range(NI):
            gci = work.tile([128, D], F32, tag="f32big")
            nc.sync.dma_start(out=gci, in_=gen_rows[ic * 128:(ic + 1) * 128, :])
            sq_scr = work.tile([128, D], BF16, tag="sqscr")
            nc.scalar.activation(sq_scr, gci, AF.Square,
                                 accum_out=a2h_col[:, ic:ic + 1])
        for ic in range(NI):
            nc.sync.dma_start(out=a2_row[0:1, ic * 128:(ic + 1) * 128],
                              in_=a2h_col[:, ic:ic + 1])
        nc.sync.dma_start(out=a2_dram[:, :], in_=a2_row)
        a2d = a2_dram[:, :]
        a2_bc_src = bass.AP(tensor=a2d.tensor, offset=a2d.offset,
                            ap=[[0, 128], a2d.ap[1]])
        nc.sync.dma_start(out=a2h_bc, in_=a2_bc_src)

        # ---- pools for the pipelined main loop ----
        kt_pool = ctx.enter_context(tc.tile_pool(name="kt_pool", bufs=NJ))
        tTb_pool = ctx.enter_context(tc.tile_pool(name="tTb_pool", bufs=2))
        tbf_pool = ctx.enter_context(tc.tile_pool(name="tbf_pool", bufs=6))
        ph1 = ExitStack()
        ps_pool = ph1.enter_context(tc.tile_pool(name="ps_pool", bufs=4,
                                                 space="PSUM"))
        mout_pool = ph1.enter_context(tc.tile_pool(name="mout_pool", bufs=1,
                                                   space="PSUM"))

        kts = {}
        tbfs = {}
        mout = {}

        def emit_out_dh0(j):
            half = 0 if j < NJG else 1
            j0 = 0 if half == 0 else NJG
            j1 = NJG - 1 if half == 0 else NJ - 1
            if j == j0:
                mout[half] = mout_pool.tile([128, NI, DH], F32, tag="mout",
                                            name=f"mout{half}")
            start = j == j0
            stop = j == j1
            for ic in range(NI):
                lhs = kts[j][:, ic * 128:(ic + 1) * 128]
                nc.tensor.matmul(mout[half][:, ic, :], lhsT=lhs,
                                 rhs=tbfs[j][:, 0:DH],
                                 start=start, stop=stop)
            if stop:
                dst = sgen if half == 0 else spos
                for ic in range(NI):
                    nc.scalar.copy(dst[:, ic, 0:DH], mout[half][:, ic, :])

        # ---- main loop over target j-tiles ----
        for g in range(NJ // NG):
            j0 = g * NG * 128
            tTb = tTb_pool.tile([128, ND, NG * 128], BF16, tag="tTb",
                                name=f"tTb{g}")
            nc.sync.dma_start(
                out=tTb,
                in_=targets_T_bf[:, j0:j0 + NG * 128].rearrange(
                    "(c p) j -> p c j", p=128))
            for k in range(NG):
                js = g * NG + k
                tbf = tbf_pool.tile([128, D], BF16, tag="tbf",
                                    name=f"tbf{js % 6}")
                tbfs[js] = tbf
                nc.sync.dma_start(out=tbf,
                                  in_=targets_bf[js * 128:(js + 1) * 128, :])
                # b2: Pool squares (SBUF only), DVE free-axis reduce
                b2_scr = work.tile([128, D], BF16, tag="sqscr")
                nc.gpsimd.tensor_tensor(out=b2_scr, in0=tbf, in1=tbf,
                                        op=ALU.mult)
                nc.vector.tensor_reduce(b2h[:, js:js + 1], b2_scr,
                                        axis=mybir.AxisListType.X,
                                        op=ALU.add)
                # distance cross-term a.b
                ps = ps_pool.tile([128, RPC], F32, tag="ps", name=f"ps{js % 3}")
                for dc in range(ND):
                    nc.tensor.matmul(ps,
                                     lhsT=tTb[:, dc, k * 128:(k + 1) * 128],
                                     rhs=genT[:, dc, :],
                                     start=(dc == 0), stop=(dc == ND - 1))
                # ps = -2*a.b + a2 (one fused DVE op), then += diag fill
                nc.vector.scalar_tensor_tensor(
                    out=ps, in0=ps, scalar=-2.0, in1=a2h_bc,
                    op0=ALU.mult, op1=ALU.add)
                off = T - js * 128
                nc.vector.tensor_tensor(out=ps, in0=ps,
                                        in1=fill_sb[:, off:off + RPC],
                                        op=ALU.add)
                if DEBUG and js == 0:
                    dscr = const.tile([128, RPC], F32, tag="d2scr")
                    nc.vector.tensor_copy(dscr, ps)
                    nc.sync.dma_start(out=d2_dbg[:, :], in_=dscr)
                # sqrt(d2 + F) with per-partition bias b2, then exp
                nc.scalar.activation(ps, ps, AF.Sqrt, bias=b2h[:, js:js + 1])
                if DEBUG and js == 0:
                    dscr2 = const.tile([128, RPC], F32, tag="distscr")
                    nc.vector.tensor_copy(dscr2, ps)
                    nc.sync.dma_start(out=dist_dbg[:, :], in_=dscr2)
                kt = kt_pool.tile([128, RPC], BF16, tag="kt", name=f"kt{js}")
                kts[js] = kt
                nc.scalar.activation(kt, ps, AF.Exp, scale=EXP_SCALE,
                                     accum_out=cs_sb[:, js:js + 1])
                if DEBUG and js == 0:
                    kscr = const.tile([128, RPC], F32, tag="ktscr")
                    nc.vector.tensor_copy(kscr, kt)
                    nc.sync.dma_start(out=kt_dbg[:, :], in_=kscr)
                if js >= LAG:
                    emit_out_dh0(js - LAG)
        for j in range(NJ - LAG, NJ):
            emit_out_dh0(j)

        nc.sync.dma_start(out=cs_part[:, :], in_=cs_sb)
        if DEBUG:
            nc.sync.dma_start(out=b2_dbg[:, :], in_=b2h)
            nc.sync.dma_start(out=a2_dbg[:, :], in_=a2h_bc)
        ph1.close()  # release phase-1 PSUM pools before phase 2's

        def combine_store(ic, dh):
            # out = beta*S_pos - alpha*S_gen, fused via scalar_tensor_tensor
            d0 = dh * DH
            t2 = work.tile([128, DH], F32, tag="cmb")
            nc.vector.tensor_scalar_mul(t2, sgen[:, ic, d0:d0 + DH],
                                        alpha[:, ic:ic + 1])
            t1 = work.tile([128, DH], F32, tag="cmb")
            nc.vector.scalar_tensor_tensor(
                out=t1, in0=spos[:, ic, d0:d0 + DH], scalar=beta[:, ic:ic + 1],
                in1=t2, op0=ALU.mult, op1=ALU.subtract)
            nc.sync.dma_start(
                out=out[ic * 128:(ic + 1) * 128, d0:d0 + DH], in_=t1)

        # ---- phase 2: dh1 output matmuls + row sums (rg: one PSUM bank
        # per accumulation group -- shared-bank groups corrupt on HW) ----
        vt_pool = ctx.enter_context(tc.tile_pool(name="vt_pool", bufs=4))
        m2_pool = ctx.enter_context(tc.tile_pool(name="m2_pool", bufs=1,
                                                 space="PSUM"))
        rg_pool = ctx.enter_context(tc.tile_pool(name="rg_pool", bufs=NI,
                                                 space="PSUM"))
        m2 = {}
        rgt = {}
        for j in range(NJ):
            half = 0 if j < NJG else 1
            j0 = 0 if half == 0 else NJG
            j1 = NJG - 1 if half == 0 else NJ - 1
            vt = vt_pool.tile([128, DH], BF16, tag="vt", name=f"vt{j % 4}")
            nc.sync.dma_start(out=vt,
                              in_=targets_bf[j * 128:(j + 1) * 128, DH:D])
            if j == j0:
                m2[half] = m2_pool.tile([128, NI, DH], F32, tag="m2",
                                        name=f"m2{half}")
                for ic in range(NI):
                    rgt[(half, ic)] = rg_pool.tile([128, 1], F32, tag="rgt",
                                                   name=f"rg{half}_{ic}")
            for ic in range(NI):
                lhs = kts[j][:, ic * 128:(ic + 1) * 128]
                nc.tensor.matmul(m2[half][:, ic, :], lhsT=lhs,
                                 rhs=vt, start=(j == j0), stop=(j == j1))
                nc.tensor.matmul(rgt[(half, ic)], lhsT=lhs, rhs=ones_bf,
                                 start=(j == j0), stop=(j == j1))
            if j == j1:
                dst = sgen if half == 0 else spos
                # beta = 1e12*rg (gen half) scales POS; alpha = 1e12*rp
                ab = beta if half == 0 else alpha
                for ic in range(NI):
                    nc.scalar.copy(dst[:, ic, DH:D], m2[half][:, ic, :])
                    nc.vector.tensor_scalar_mul(ab[:, ic:ic + 1],
                                                rgt[(half, ic)], INV_NORM2)
        rs_sb = const.tile([128, NI], F32, tag="rs_sb")
        nc.vector.tensor_add(rs_sb, alpha, beta)
        nc.vector.tensor_scalar_mul(rs_sb, rs_sb, 1.0 / INV_NORM2)
        nc.sync.dma_start(out=rs_out[:, :], in_=rs_sb)
        for ic in range(NI):
            combine_store(ic, 0)
            combine_store(ic, 1)

    nc.compile()
    return nc


def make_in_maps(gen, pos, G, P, D):
    import ml_dtypes
    T = G + P
    RPC = G // NCORES
    WFILL = T + RPC
    targets = np.concatenate([gen, pos], axis=0).astype(np.float32)
    targets_bf = np.ascontiguousarray(targets.astype(ml_dtypes.bfloat16))
    targets_t_bf = np.ascontiguousarray(targets_bf.T)
    in_maps = []
    p = np.arange(128)
    for c in range(NCORES):
        fill = np.zeros((128, WFILL), ml_dtypes.bfloat16)
        # diagonal entries: q = T + p - c*RPC  (j-tile independent; the
        # per-tile window [T - js*128, +RPC) hits exactly the gen diagonal)
        q = T + p - c * RPC
        fill[p, q] = BIGF
        gen_c = np.ascontiguousarray(gen[c * RPC:(c + 1) * RPC]).astype(np.float32)
        genT_bf = np.ascontiguousarray(gen_c.astype(ml_dtypes.bfloat16).T)
        in_maps.append({
            "gen_rows": gen_c,
            "genT_bf": genT_bf,
            "targets_bf": targets_bf,
            "targets_T_bf": targets_t_bf,
            "fill_wide": fill,
        })
    return in_maps


def _exact_numpy_reference(gen, pos):
    """Bit-faithful (float64) fallback for inputs outside the clamped regime."""
    G, D = gen.shape
    gen64 = gen.astype(np.float64)
    pos64 = pos.astype(np.float64)
    tgt = np.concatenate([gen64, pos64], 0)
    d2 = (gen64 * gen64).sum(-1)[:, None] + (tgt * tgt).sum(-1)[None, :] \
        - 2.0 * gen64 @ tgt.T
    dist = np.sqrt(np.maximum(d2, 0.0))
    if D > 10:
        dist = dist / np.sqrt(D)
    idx = np.arange(G)
    dist[idx, idx] = 1e6
    k = np.exp(-dist / TEMP)
    rs = k.sum(-1, keepdims=True)
    cs = k.sum(-2, keepdims=True)
    nk = k / np.sqrt(np.maximum(rs * cs, CLAMP))
    pos_c = nk[:, G:] * nk[:, :G].sum(-1, keepdims=True)
    neg_c = nk[:, :G] * nk[:, G:].sum(-1, keepdims=True)
    return (pos_c @ pos64 - neg_c @ gen64).astype(np.float32)


_NC_CACHE = {}


def _get_nc(G, P, D):
    key = (G, P, D)
    if key not in _NC_CACHE:
        _NC_CACHE[key] = build_nc(G, P, D)
    return _NC_CACHE[key]


def kernel(gen_features, pos_features):
    global LAST_RESULT
    gen = np.asarray(gen_features, dtype=np.float32)
    pos = np.asarray(pos_features, dtype=np.float32)
    G, D = gen.shape
    P = pos.shape[0]
    nc = _get_nc(G, P, D)
    in_maps = make_in_maps(gen, pos, G, P, D)
    res = run_bass_kernel_spmd(nc, in_maps, core_ids=list(range(NCORES)),
                               trace=TRACE)
    LAST_RESULT = res
    out = np.concatenate([res.results[c]["out"] for c in range(NCORES)], axis=0)

    # Clamp-regime guard: the device kernel assumes rs_i*cs_j <= 1e-12
    # everywhere (always true for this problem's data). Verify from the
    # device's own row/column sums; fall back to exact evaluation if not.
    cs_glob = sum(res.results[c]["cs_part"] for c in range(NCORES))
    rs_max = max(float(res.results[c]["rs_out"].max()) for c in range(NCORES))
    if rs_max * float(cs_glob.max()) > 0.25 * CLAMP:
        return _exact_numpy_reference(gen, pos)
    return out.astype(np.float32)
